# revision 1
# baseline (speedup 1.0000x reference)
"""ColorHistogramLoss Trainium2 kernel (8 NeuronCores, data-parallel).

Strategy: shard batch (32 -> 4 per core); each core streams its 25MB of
pixels through SBUF as 8 iterations of [128, 2048] plane-triples (4 real +
4 fake) and produces cumulative histogram-edge counts which the host
differences into the three 10-bin histograms and the scalar loss.

Engine split (both ~95% busy, VectorE-bound at ~420us):
- VectorE (22 fp32 passes/iter): channel diffs u/v/w; d = max|diffs| and
  1/d via fused custom-DVE ops; case masks mb = "blue is max" and
  mg' = "green is max, not blue" as single fused sign-test ops with free
  count accumulators; per-case shifted hue values A2/B2/C2 (out-of-case
  pixels pushed +-8 out of edge range, so hue edge counts need no select
  chain); saturation edge counts as dual-edge fused compares
  #{c*mx > d} packed two counts per f32 accumulator (cntA + 4096*cntB).
- ScalarE (19 activations/iter): hue + value edge counts as
  Sign(x - edge) with fused accumulation; host decodes
  N_lt = (N - sum_sign)/2.  The last iteration's value masks run on
  VectorE instead so ScalarE is not the pipeline tail.

All on-device count arithmetic is exact in f32; histogram differencing
and the weighted mean run on host (tiny).  Total rel err ~5e-5 vs the
f32 reference (boundary-ulp effects from approximate 1/d only).
"""

import sys

if "/opt/trn_rl_repo" not in sys.path:
    sys.path.insert(0, "/opt/trn_rl_repo")

import numpy as np

from concourse import bacc, mybir, tile
from concourse import bass_utils

# ---- problem constants (hardcoded; kernel.py must be self-contained) ----
B, C, H, W = 32, 3, 512, 512
NCORES = 8
BPC = B // NCORES            # batches per core
P, F = 128, 2048             # SBUF tile: one [512,512] plane = [128, 2048]
NITER = 2 * BPC              # 4 real + 4 fake plane-triple iterations
NEDGE = 26                   # acc slots: 12 hue-case + 9 val + 4 sat-dual + 1 sat
ACCW = 32                    # padded accumulator width
NPIX = B * H * W             # pixels per full histogram
ALPHA, BETA, GAMMA = 0.3, 0.4, 0.4

AF = mybir.AluOpType
F32 = mybir.dt.float32

LAST_EXEC_NS = None
_CACHE = {}

PACK = 4096.0  # EDGE2* dual-count packing: accum = cntA + PACK*cntB (exact in f32)

# Hue edge counting runs on per-case shifted values (shift=8 keeps ulp tiny):
#   A2 = hA + 8*(mb+mg')  (r-case in range, others at ~[7,9])
#   B2 = hB - 8*mg'       (g-case at [-9,-7], others in [-1,1])
#   C2 = hC - 8*mb        (b-case at [-9,-7], others in [-1,1])
# where hA=u/d, hB=v/d, hC=w/d.  Case totals R, G come free from the
# mask-op accumulators.  Slot edges (sign-counted on ScalarE):
HUE_EDGES = (
    -0.6, 0.0, 0.6,                 # A2: NA(-0.6), NEG, NA(0.6)
    -8.8, -8.2, -7.6, -7.0,         # B2: NB(e-10) for e=1.2..3.0
    -8.4, -7.8, -7.2,               # C2: NC(e-12) for e=3.6..4.8
)


def _register_custom_ops():
    """Author + register fused DVE ops in the dve_ops registry at runtime
    (the repo list is read-only; registration is by-name so appending to the
    module-level OPS list is sufficient for table-gen and tracing)."""
    from concourse import dve_ops
    from concourse.dve_spec import (
        C0, C1, C2, Spec, Src0, Src1, Zero, _has_src1, lower, maxx,
    )
    from concourse.dve_uop import DveOpSpec

    if hasattr(dve_ops, "HUE_MOD6"):
        return dve_ops

    _y = Src0 * Src1

    def _ref_hue_mod6(in0, in1, c0, c1, c2):
        y = in0.astype(np.float32) * in1
        return (y + c0 * (y < 0)).astype(np.float32)

    def _ref_abs2max(in0, in1, c0, c1, c2):
        return np.maximum(np.abs(in0.astype(np.float32)), np.abs(in1)).astype(
            np.float32
        )

    def _ref_absmax3(in0, in1, c0, c1, c2):
        return np.maximum(in0.astype(np.float32), np.abs(in1)).astype(np.float32)

    def _ref_edge2d(in0, in1, c0, c1, c2):
        b = ((in0.astype(np.float32) * c0 > in1) + c1 * (in0 * c2 > in1)).astype(
            np.float32
        )
        return b, b.reshape(b.shape[0], -1).sum(axis=-1, keepdims=True)

    from operator import add as _add

    defs = [
        # out = y + c0*(y<0), y = in0*in1   (hue mod-6 wrap, fused)
        ("HUE_MOD6", Spec(body=_y + C0 * (_y < Zero), reference=_ref_hue_mod6)),
        # out = max(|in0|, |in1|)
        (
            "ABS2MAX",
            Spec(
                body=maxx(maxx(Src0, Zero - Src0), maxx(Src1, Zero - Src1)),
                reference=_ref_abs2max,
            ),
        ),
        # out = max(in0, |in1|)
        (
            "ABSMAX3",
            Spec(
                body=maxx(Src0, maxx(Src1, Zero - Src1)),
                reference=_ref_absmax3,
            ),
        ),
        # dual sat-edge count: accum = #{in0*c0 > in1} + c1*#{in0*c2 > in1}
        (
            "EDGE2D",
            Spec(
                body=(Src0 * C0 > Src1) + C1 * ((Src0 * C2) > Src1),
                accum=_add,
                accum_init=Zero,
                reference=_ref_edge2d,
            ),
        ),
        # dual edge count: accum = #{in0 < c0} + c1*#{in0 < c2}
        (
            "EDGE2",
            Spec(
                body=(Src0 < C0) + C1 * (Src0 < C2),
                accum=_add,
                accum_init=Zero,
                reference=lambda in0, in1, c0, c1, c2: (
                    lambda b: (b, b.reshape(b.shape[0], -1).sum(-1, keepdims=True))
                )(((in0 < c0) + c1 * (in0 < c2)).astype(np.float32)),
            ),
        ),
        # mb = (in0 >= 0) & (in1 <= 0); accum = count  (in0=v, in1=u)
        (
            "MBC",
            Spec(
                body=(Src0 >= Zero) & (Src1 <= Zero),
                accum=_add,
                accum_init=Zero,
                reference=lambda in0, in1, c0, c1, c2: (
                    lambda b: (b, b.reshape(b.shape[0], -1).sum(-1, keepdims=True))
                )(((in0 >= 0) & (in1 <= 0)).astype(np.float32)),
            ),
        ),
        # nmg = -[(in0 > 0) & (in1 <= 0)]; accum = -count  (in0=u, in1=w)
        (
            "NMGC",
            Spec(
                body=Zero - ((Src0 > Zero) & (Src1 <= Zero)),
                accum=_add,
                accum_init=Zero,
                reference=lambda in0, in1, c0, c1, c2: (
                    lambda b: (b, b.reshape(b.shape[0], -1).sum(-1, keepdims=True))
                )((-((in0 > 0) & (in1 <= 0))).astype(np.float32)),
            ),
        ),
    ]
    for name, spec in defs:
        row = 1 + len(dve_ops.OPS)
        shas = {}
        for ver in ("v3", "v4"):
            uops = lower(spec, ver=ver)
            shas[ver] = DveOpSpec(
                name=name, opcode=row, uops=uops, rd1_en=_has_src1(spec)
            ).sha(ver)
        op = dve_ops.DveOp(name, spec, False, uops_sha=shas)
        dve_ops.OPS.append(op)
        dve_ops.CUSTOM_DVE_SPECS[name] = spec
        dve_ops._SUB_OPCODE_FOR_NAME[name] = row
        setattr(dve_ops, name, op)
    return dve_ops


def _build():
    dve_ops = _register_custom_ops()
    nc = bacc.Bacc(
        "TRN2", target_bir_lowering=False, debug=False, num_devices=NCORES
    )
    xr = nc.dram_tensor("x_real", [BPC * C * P, F], F32, kind="ExternalInput").ap()
    xf = nc.dram_tensor("x_fake", [BPC * C * P, F], F32, kind="ExternalInput").ap()
    out = nc.dram_tensor("out", [NITER * P, ACCW], F32, kind="ExternalOutput").ap()

    with tile.TileContext(nc) as tc:
        with tc.tile_pool(name="main", bufs=2) as io_pool, tc.tile_pool(
            name="tmp", bufs=1
        ) as tmp_pool:
            # per-edge bias tiles for ScalarE Sign activations (bias = -edge)
            ebias = []
            for k in range(1, 10):
                bt = tmp_pool.tile([P, 1], F32, tag=f"eb{k}", name=f"eb{k}")
                nc.gpsimd.memset(bt[:], -(0.1 * k))
                ebias.append(bt)
            # hue case-edges on shifted per-case values A2/B2/C2
            hedges = HUE_EDGES
            hbias = []
            for idx, e in enumerate(hedges):
                ht = tmp_pool.tile([P, 1], F32, tag=f"hb{idx}", name=f"hb{idx}")
                nc.gpsimd.memset(ht[:], -e)
                hbias.append(ht)
            for it in range(NITER):
                src = xr if it < BPC else xf
                bi = it % BPC

                def plane(c):
                    q = bi * C + c
                    return src[q * P : (q + 1) * P, :]

                r = io_pool.tile([P, F], F32, tag="r")
                g = io_pool.tile([P, F], F32, tag="g")
                bl = io_pool.tile([P, F], F32, tag="bl")
                # bl, r first: the opening VectorE op (v = bl - r) needs them
                nc.sync.dma_start(bl[:], plane(2))
                nc.sync.dma_start(r[:], plane(0))
                nc.sync.dma_start(g[:], plane(1))

                # double-buffer the tiles ScalarE reads across iterations
                # (mx=t1, A2=t2, B2=t4, C2=t5, d=t10) to break WAR stalls
                t = [
                    tmp_pool.tile(
                        [P, F], F32, tag=f"t{i}", name=f"t{i}",
                        bufs=2 if i in (1, 2, 4, 5, 10) else 1,
                    )
                    for i in range(11)
                ]
                V = nc.vector

                u = t[2]
                V.tensor_tensor(u[:], g[:], bl[:], AF.subtract)
                v = t[4]
                V.tensor_tensor(v[:], bl[:], r[:], AF.subtract)
                w = t[5]
                V.tensor_tensor(w[:], r[:], g[:], AF.subtract)
                # d = mx - mn == max(|u|, |v|, |w|) (exact: same fl-subtracts)
                d2 = t[3]
                V._custom_dve(dve_ops.ABS2MAX, out=d2[:], in0=u[:], in1=v[:])
                d = t[10]
                V._custom_dve(dve_ops.ABSMAX3, out=d[:], in0=d2[:], in1=w[:])
                rd = t[3]
                V.reciprocal_approx_fast(rd[:], d[:])
                acc = io_pool.tile([P, 19], F32, tag="acc")
                accv = io_pool.tile([P, 7], F32, tag="accv")
                scr = t[9]
                scr2 = tmp_pool.tile([P, F], F32, tag="scr2", name="scr2")
                SIGN = mybir.ActivationFunctionType.Sign
                mb = t[6]
                # mb = (v>=0)&(u<=0) == (mx==bl); accum -> count(mb)
                V._custom_dve(
                    dve_ops.MBC, out=mb[:], in0=v[:], in1=u[:],
                    accum_out=accv[:, 5:6],
                )
                nmg = t[8]
                # nmg = -[(u>0)&(w<=0)] == -[mg & !mb]; accum -> -count(mg')
                V._custom_dve(
                    dve_ops.NMGC, out=nmg[:], in0=u[:], in1=w[:],
                    accum_out=accv[:, 6:7],
                )
                s8 = t[7]
                V.tensor_tensor(s8[:], mb[:], nmg[:], AF.subtract)  # mb + mg'
                hA = t[9]
                V.tensor_tensor(hA[:], u[:], rd[:], AF.mult)
                A2 = t[2]
                V.scalar_tensor_tensor(A2[:], s8[:], 8.0, hA[:], AF.mult, AF.add)
                hBp = t[9]
                V.tensor_tensor(hBp[:], v[:], rd[:], AF.mult)
                B2 = t[4]
                V.scalar_tensor_tensor(B2[:], nmg[:], 8.0, hBp[:], AF.mult, AF.add)
                hCp = t[9]
                V.tensor_tensor(hCp[:], w[:], rd[:], AF.mult)
                C2 = t[5]
                V.scalar_tensor_tensor(C2[:], mb[:], -8.0, hCp[:], AF.mult, AF.add)
                # mx late: only the val/sat masks consume it
                m1, mx = t[0], t[1]
                V.tensor_tensor(m1[:], r[:], g[:], AF.max)
                V.tensor_tensor(mx[:], m1[:], bl[:], AF.max)
                # hue case-edge counts on ScalarE, sign-style: slots 0..9
                # accum = sum(Sign(x - e)); host decodes N_lt = (N - S)/2
                case_tiles = [A2] * 3 + [B2] * 4 + [C2] * 3
                for idx in range(10):
                    nc.scalar.activation(
                        scr2[:], case_tiles[idx][:], SIGN, bias=hbias[idx][:],
                        accum_out=acc[:, idx : idx + 1],
                    )
                if it < NITER - 1:
                    # val masks on ScalarE: slots 10..18 (sign-style)
                    for k in range(1, 10):
                        nc.scalar.activation(
                            scr2[:], mx[:], SIGN, bias=ebias[k - 1][:],
                            accum_out=acc[:, 9 + k : 10 + k],
                        )
                else:
                    # last iteration: run val masks on VectorE (EDGE2 duals)
                    # so ScalarE isn't the pipeline tail. Direct counts,
                    # flagged for the host by writing them as negatives
                    # minus one... (decoded by slot style table instead)
                    for j in range(4):
                        V._custom_dve(
                            dve_ops.EDGE2,
                            out=scr[:],
                            in0=mx[:],
                            s0=0.1 * (2 * j + 1),
                            s1=PACK,
                            imm2=0.1 * (2 * j + 2),
                            accum_out=acc[:, 10 + j : 11 + j],
                        )
                    V.tensor_scalar(
                        scr[:], mx[:], 0.9, None, AF.is_lt, AF.add,
                        accum_out=acc[:, 14:15],
                    )
                # sat masks on VectorE: dual-edge fused counts, accv 0..3
                # slot = #{0.1(2j+1)*mx > d} + PACK * #{0.1(2j+2)*mx > d}
                for j in range(4):
                    V._custom_dve(
                        dve_ops.EDGE2D,
                        out=scr[:],
                        in0=mx[:],
                        in1=d[:],
                        s0=0.1 * (2 * j + 1),
                        s1=PACK,
                        imm2=0.1 * (2 * j + 2),
                        accum_out=accv[:, j : j + 1],
                    )
                # 9th sat edge: direct single count, accv 4
                V.scalar_tensor_tensor(
                    scr[:], mx[:], 0.9, d[:], AF.mult, AF.is_gt,
                    accum_out=accv[:, 4:5],
                )
                nc.sync.dma_start(out[it * P : (it + 1) * P, 0:19], acc[:, :])
                nc.sync.dma_start(out[it * P : (it + 1) * P, 19:26], accv[:, :])

    nc.compile()
    return nc


def _register_ntff_hook():
    """Register the axon NTFF profiling hook (the container's antenv stub
    lacks axon_hooks, so trn_boot's registration was skipped). Also keep
    profile artifacts local instead of uploading to a share."""
    import types

    import antenv

    if "antenv.axon_hooks" not in sys.modules:
        mod = types.ModuleType("antenv.axon_hooks")
        holder = [None]
        mod.set_axon_ntff_profile_hook = lambda h: holder.__setitem__(0, h)
        mod.get_axon_ntff_profile_hook = lambda: holder[0]
        sys.modules["antenv.axon_hooks"] = mod
        antenv.axon_hooks = mod
    from antenv import axon_hooks

    if axon_hooks.get_axon_ntff_profile_hook() is None:
        from trn_agent_boot.trn_boot import _ntff_profile_via_ctypes

        axon_hooks.set_axon_ntff_profile_hook(
            _ntff_profile_via_ctypes("/opt/axon/libaxon_pjrt.so")
        )
    bass_utils.upload_artifacts = lambda tmpdir: tmpdir


def _get_nc():
    if "nc" not in _CACHE:
        _CACHE["nc"] = _build()
    return _CACHE["nc"]


def kernel(x_real: np.ndarray, x_fake: np.ndarray) -> np.ndarray:
    global LAST_EXEC_NS
    nc = _get_nc()

    in_maps = []
    for c in range(NCORES):
        sl = slice(c * BPC, (c + 1) * BPC)
        in_maps.append(
            {
                "x_real": np.ascontiguousarray(x_real[sl]).reshape(BPC * C * P, F),
                "x_fake": np.ascontiguousarray(x_fake[sl]).reshape(BPC * C * P, F),
            }
        )

    import os

    trace = bool(int(os.environ.get("KERNEL_TRACE", "0")))
    if trace:
        _register_ntff_hook()
    res = bass_utils.run_bass_kernel_spmd(
        nc, in_maps, core_ids=list(range(NCORES)), trace=trace
    )
    LAST_EXEC_NS = res.exec_time_ns
    _CACHE["last_res"] = res

    # Cols 0:10 hue case-edges + 10:19 val: sign-sums S = cnt_gt - cnt_lt,
    # decoded as N_lt = (N - S)/2.  Cols 19:23: packed dual sat counts
    # cntA + PACK*cntB (direct C_lt); col 23: sat edge 9; col 24: count(mb);
    # col 25: -count(mg').
    sign_sums = np.zeros((2, 19), np.float64)
    sat_C = np.zeros((2, 9), np.float64)
    val_direct = np.zeros(9, np.float64)  # last iteration's val counts (on V)
    caseB = np.zeros(2)
    caseG = np.zeros(2)
    for core_out in res.results:
        o = np.asarray(core_out["out"]).reshape(NITER, P, ACCW)
        for t_idx, sl in ((0, slice(0, BPC)), (1, slice(BPC, NITER))):
            blk = o[sl]
            sign_sums[t_idx, :10] += blk[:, :, :10].sum(axis=(0, 1))
            if t_idx == 0:
                sign_sums[0, 10:19] += blk[:, :, 10:19].sum(axis=(0, 1))
            else:
                # fake: iters 4..6 sign-style; iter 7 direct EDGE2-packed
                sign_sums[1, 10:19] += blk[:-1, :, 10:19].sum(axis=(0, 1))
                vp = blk[-1, :, 10:14].astype(np.int64)
                val_direct[0:8:2] += (vp % int(PACK)).sum(axis=0)
                val_direct[1:8:2] += (vp // int(PACK)).sum(axis=0)
                val_direct[8] += blk[-1, :, 14].sum()
            packed = blk[:, :, 19:23].astype(np.int64)  # exact ints in f32
            sat_C[t_idx, 0:8:2] += (packed % int(PACK)).sum(axis=(0, 1))
            sat_C[t_idx, 1:8:2] += (packed // int(PACK)).sum(axis=(0, 1))
            sat_C[t_idx, 8] += blk[:, :, 23].sum()
            caseB[t_idx] += blk[:, :, 24].sum()
            caseG[t_idx] -= blk[:, :, 25].sum()
    NL = (NPIX - sign_sums) / 2.0  # [2, 19] lt-counts per slot
    # fake val: sign part covers only (BPC-1)/BPC of the pixels
    NL[1, 10:19] = (NPIX * (BPC - 1) / BPC - sign_sums[1, 10:19]) / 2.0 + val_direct
    NA, NB, NC = NL[:, 0:3], NL[:, 3:7], NL[:, 7:10]
    NEG = NA[:, 1]
    R = NPIX - caseB - caseG
    C_lt = np.zeros((2, 3, 9), np.float64)
    # hue: reassemble cumulative counts from per-case counts
    C_lt[:, 0, 0] = NA[:, 2] - NEG                       # C(0.6)
    for j in range(4):                                   # C(1.2)..C(3.0)
        C_lt[:, 0, 1 + j] = (R - NEG) + NB[:, j]
    for j in range(3):                                   # C(3.6)..C(4.8)
        C_lt[:, 0, 5 + j] = (R - NEG) + caseG + NC[:, j]
    C_lt[:, 0, 8] = (NPIX - NEG) + NA[:, 0]              # C(5.4)
    C_lt[:, 2, :] = NL[:, 10:19]                         # val
    C_lt[:, 1, :] = sat_C

    hist = np.zeros((2, 3, 10), np.float64)
    hist[:, :, 0] = C_lt[:, :, 0]
    hist[:, :, 1:9] = C_lt[:, :, 1:] - C_lt[:, :, :-1]
    hist[:, :, 9] = NPIX - C_lt[:, :, 8]

    dmean = np.abs(hist[0] - hist[1]).mean(axis=1)   # [3] = h, s, v
    loss = ALPHA * dmean[0] + BETA * dmean[1] + GAMMA * dmean[2]
    return np.asarray(loss, dtype=np.float32)



# revision 4
# speedup vs baseline: 1.1511x; 1.1511x over previous
"""ColorHistogramLoss Trainium2 kernel (8 NeuronCores, data-parallel).

Strategy: shard batch (32 -> 4 per core); each core streams its 8 images
(4 real + 4 fake) as [128, 2048] plane-triples and produces direct
cumulative histogram-edge counts; the host reassembles the three 10-bin
histograms per source and computes the scalar loss.

v2 design (VectorE+ScalarE balanced; GPSIMD unusable - SBUF port
contention with VectorE inflates both ~2.8x):
- VectorE (19 passes/iter): v=b-r, w=r-g, u=g-b, m1, mx; fused customs
  d=max(|v|,|w|,|v+w|) (1 pass), s8=mb+mg from (v,w) (1 pass, 8 ALU
  slices), case-shift customs v'=v+BIG*(1-mg), w'=w+BIG*(1-mb) straight
  from (v,w); u'=u+BIG*s8 (STT); rd=1/d; q=mx*rd.  Hue edge counts as
  dual multiply-compares (u'|v'|w')*rd vs case edges - out-of-case
  pixels sit at ~BIG*rd >= BIG-1, outside every edge, so counts are
  exactly per-case.  Two val dual-edge counts (packed c_lo+4096*c_hi).
- ScalarE (17 Sign activations/iter): val mx @ {0.5..0.9}, sat
  q=mx/d @ {10/k} (sat<c <=> q>10/k), and three mul-free hue counts:
  NEG=#{u'<0}, G-total=#{v'<2}, B-total=#{w'<2}.

Counts are exact in f32 (recip approx gives ~1e-7 slivers only); host
decode uses direct per-case cumulative counts.
"""

import sys

if "/opt/trn_rl_repo" not in sys.path:
    sys.path.insert(0, "/opt/trn_rl_repo")

import numpy as np

from concourse import bacc, mybir, tile
from concourse import bass_utils

# ---- problem constants (hardcoded; kernel.py must be self-contained) ----
B, C, H, W = 32, 3, 512, 512
NCORES = 8
BPC = B // NCORES            # batches per core
P, F = 128, 2048             # SBUF tile: one [512,512] plane = [128, 2048]
NITER = 2 * BPC              # 4 real + 4 fake plane-triple iterations
ACCW = 32                    # padded accumulator width
NPIX = B * H * W             # pixels per full histogram (all cores)
ALPHA, BETA, GAMMA = 0.3, 0.4, 0.4

AF = mybir.AluOpType
F32 = mybir.dt.float32

LAST_EXEC_NS = None
_CACHE = {}

PACK = 4096.0   # dual-count packing: accum = cnt_lo + PACK*cnt_hi (exact f32)
BIG = 1048576.0  # out-of-case shift; BIG*rd >= BIG-1 >> all hue edges

# ScalarE sign-counted edges
S_MX = (0.5, 0.6, 0.7, 0.8, 0.9)            # val: count mx < e  (slots 10-14)
S_Q = tuple(10.0 / k for k in range(1, 10))  # sat: count q > e   (slots 15-23)

# acc slot map (see build)
NSLOT = 24


def _register_custom_ops():
    """Author + register fused DVE ops in the dve_ops registry at runtime."""
    from concourse import dve_ops
    from concourse.dve_spec import (
        C0, C1, C2, One, Spec, Src0, Src1, Zero, _has_src1, lower, maxx, minn,
    )
    from concourse.dve_uop import DveOpSpec
    from operator import add as _add

    if hasattr(dve_ops, "D3X"):
        return dve_ops

    def cref(f):
        def r(in0, in1, c0, c1, c2):
            b = f(in0, in1, c0, c1, c2).astype(np.float32)
            return b, b.reshape(b.shape[0], -1).sum(-1, keepdims=True)
        return r

    _t = Src0 + Src1

    def _mg(v, w):
        return ((v + w) < 0) & (w <= 0)

    def _mb(v, w):
        return (v >= 0) & ((v + w) >= 0)

    defs = [
        # d = max(|v|,|w|,|v+w|) == mx-mn (exact: same fl subtract results)
        ("D3X", Spec(
            body=maxx(maxx(maxx(Src0, Src1), _t), Zero - minn(minn(Src0, Src1), _t)),
            reference=lambda in0, in1, c0, c1, c2: np.maximum(
                np.maximum(np.abs(in0), np.abs(in1)), np.abs(in0 + in1)
            ).astype(np.float32),
        )),
        # s8 = mb + mg from (v,w): mb=(v>=0)&(v+w>=0), mg=(v+w<0)&(w<=0)
        ("S8C", Spec(
            body=((Src0 >= Zero) & (_t >= Zero)) + ((_t < Zero) & (Src1 <= Zero)),
            reference=lambda in0, in1, c0, c1, c2: (
                _mb(in0, in1) + _mg(in0, in1)
            ).astype(np.float32),
        )),
        # v' = v + C0*(1 - mg(v,w))
        ("VSH", Spec(
            body=Src0 + C0 * (One - ((_t < Zero) & (Src1 <= Zero))),
            reference=lambda in0, in1, c0, c1, c2: (
                in0 + c0 * (1.0 - _mg(in0, in1))
            ).astype(np.float32),
        )),
        # w' = w + C0*(1 - mb(v,w))   (Src0=v, Src1=w)
        ("WSH", Spec(
            body=Src1 + C0 * (One - ((Src0 >= Zero) & (_t >= Zero))),
            reference=lambda in0, in1, c0, c1, c2: (
                in1 + c0 * (1.0 - _mb(in0, in1))
            ).astype(np.float32),
        )),
        # dual mult-compare count: accum = #{a*b < C0} + C1*#{a*b < C2}
        ("MULCMP", Spec(
            body=(Src0 * Src1 < C0) + C1 * ((Src0 * Src1) < C2),
            accum=_add, accum_init=Zero,
            reference=cref(lambda a, b, c0, c1, c2: (
                ((a * b).astype(np.float32) < c0)
                + c1 * ((a * b).astype(np.float32) < c2)
            )),
        )),
        # single mult-compare count: accum = #{a*b < C0}
        ("MULCMP1", Spec(
            body=(Src0 * Src1 < C0) + Zero,
            accum=_add, accum_init=Zero,
            reference=cref(lambda a, b, c0, c1, c2: (
                ((a * b).astype(np.float32) < c0).astype(np.float32)
            )),
        )),
        # dual edge count: accum = #{x < C0} + C1*#{x < C2}
        ("LT2", Spec(
            body=(Src0 < C0) + C1 * (Src0 < C2),
            accum=_add, accum_init=Zero,
            reference=cref(lambda a, b, c0, c1, c2: (a < c0) + c1 * (a < c2)),
        )),
    ]
    for name, spec in defs:
        row = 1 + len(dve_ops.OPS)
        shas = {}
        for ver in ("v3", "v4"):
            uops = lower(spec, ver=ver)
            shas[ver] = DveOpSpec(
                name=name, opcode=row, uops=uops, rd1_en=_has_src1(spec)
            ).sha(ver)
        op = dve_ops.DveOp(name, spec, False, uops_sha=shas)
        dve_ops.OPS.append(op)
        dve_ops.CUSTOM_DVE_SPECS[name] = spec
        dve_ops._SUB_OPCODE_FOR_NAME[name] = row
        setattr(dve_ops, name, op)
    return dve_ops


def _build():
    dve_ops = _register_custom_ops()
    nc = bacc.Bacc(
        "TRN2", target_bir_lowering=False, debug=False, num_devices=NCORES
    )
    xr = nc.dram_tensor("x_real", [BPC * C * P, F], F32, kind="ExternalInput").ap()
    xf = nc.dram_tensor("x_fake", [BPC * C * P, F], F32, kind="ExternalInput").ap()
    out = nc.dram_tensor("out", [NITER * P, ACCW], F32, kind="ExternalOutput").ap()

    SIGN = mybir.ActivationFunctionType.Sign

    with tile.TileContext(nc) as tc:
        with tc.tile_pool(name="main", bufs=2) as io_pool, tc.tile_pool(
            name="tmp", bufs=1
        ) as tmp_pool:
            V, S = nc.vector, nc.scalar
            # bias tiles for ScalarE sign counts (bias = -edge)
            sbias = []
            for idx, e in enumerate(S_MX + S_Q + (0.0, 2.0)):
                bt = tmp_pool.tile([P, 1], F32, tag=f"sb{idx}", name=f"sb{idx}")
                nc.gpsimd.memset(bt[:], -e)
                sbias.append(bt)
            b_mx = sbias[0:5]
            b_q = sbias[5:14]
            b_0, b_2 = sbias[14], sbias[15]

            for it in range(NITER):
                src = xr if it < BPC else xf
                bi = it % BPC

                def plane(c):
                    qI = bi * C + c
                    return src[qI * P : (qI + 1) * P, :]

                r = io_pool.tile([P, F], F32, tag="r")
                g = io_pool.tile([P, F], F32, tag="g")
                bl = io_pool.tile([P, F], F32, tag="bl")
                # b, r first: the opening VectorE op (v = bl - r) needs them
                nc.sync.dma_start(bl[:], plane(2))
                nc.sync.dma_start(r[:], plane(0))
                nc.sync.dma_start(g[:], plane(1))

                def T(tag, bufs=1):
                    return tmp_pool.tile([P, F], F32, tag=tag, name=tag, bufs=bufs)

                acc = io_pool.tile([P, NSLOT], F32, tag="acc")
                v = T("v")
                w = T("w")
                u = T("u")
                m1 = T("m1")
                mx = T("mx", bufs=2)
                d = T("d")
                rd = T("rd")
                s8 = T("s8")
                u2 = T("u2", bufs=2)
                v2 = T("v2", bufs=2)
                w2 = T("w2", bufs=2)
                q = T("q", bufs=2)
                scr = T("scr")
                scr2 = T("scr2")

                V.tensor_tensor(v[:], bl[:], r[:], AF.subtract)
                V.tensor_tensor(w[:], r[:], g[:], AF.subtract)
                V.tensor_tensor(m1[:], r[:], g[:], AF.max)
                V.tensor_tensor(u[:], g[:], bl[:], AF.subtract)
                V.tensor_tensor(mx[:], m1[:], bl[:], AF.max)
                V._custom_dve(dve_ops.D3X, out=d[:], in0=v[:], in1=w[:])
                V.reciprocal_approx_fast(rd[:], d[:])
                V._custom_dve(dve_ops.S8C, out=s8[:], in0=v[:], in1=w[:])
                V.scalar_tensor_tensor(u2[:], s8[:], BIG, u[:], AF.mult, AF.add)
                V._custom_dve(dve_ops.VSH, out=v2[:], in0=v[:], in1=w[:], s0=BIG)
                V._custom_dve(dve_ops.WSH, out=w2[:], in0=v[:], in1=w[:], s0=BIG)
                V.tensor_tensor(q[:], mx[:], rd[:], AF.mult)

                # --- ScalarE sign counts (start on mx asap) ---
                for k in range(5):
                    S.activation(scr2[:], mx[:], SIGN, bias=b_mx[k][:],
                                 accum_out=acc[:, 10 + k : 11 + k])

                # --- VectorE dual counts ---
                def mc(src0, src1, e1, e2, slot):
                    V._custom_dve(dve_ops.MULCMP, out=scr[:], in0=src0[:],
                                  in1=src1[:], s0=e1, s1=PACK, imm2=e2,
                                  accum_out=acc[:, slot : slot + 1])

                mc(u2, rd, -0.6, 0.6, 0)
                mc(v2, rd, -0.8, -0.2, 1)
                mc(v2, rd, 0.4, 1.0, 2)
                mc(w2, rd, -0.4, 0.2, 3)
                V._custom_dve(dve_ops.MULCMP1, out=scr[:], in0=w2[:], in1=rd[:],
                              s0=0.8, accum_out=acc[:, 4:5])
                V._custom_dve(dve_ops.LT2, out=scr[:], in0=mx[:], s0=0.1,
                              s1=PACK, imm2=0.2, accum_out=acc[:, 5:6])
                V._custom_dve(dve_ops.LT2, out=scr[:], in0=mx[:], s0=0.3,
                              s1=PACK, imm2=0.4, accum_out=acc[:, 6:7])

                # --- remaining ScalarE sign counts ---
                for k in range(9):
                    S.activation(scr2[:], q[:], SIGN, bias=b_q[k][:],
                                 accum_out=acc[:, 15 + k : 16 + k])
                S.activation(scr2[:], u2[:], SIGN, bias=b_0[:],
                             accum_out=acc[:, 7:8])
                S.activation(scr2[:], v2[:], SIGN, bias=b_2[:],
                             accum_out=acc[:, 8:9])
                S.activation(scr2[:], w2[:], SIGN, bias=b_2[:],
                             accum_out=acc[:, 9:10])

                nc.sync.dma_start(out[it * P : (it + 1) * P, 0:NSLOT], acc[:, :])

    nc.compile()
    return nc


def _register_ntff_hook():
    """Register the axon NTFF profiling hook; keep artifacts local."""
    import types

    import antenv

    if "antenv.axon_hooks" not in sys.modules:
        mod = types.ModuleType("antenv.axon_hooks")
        holder = [None]
        mod.set_axon_ntff_profile_hook = lambda h: holder.__setitem__(0, h)
        mod.get_axon_ntff_profile_hook = lambda: holder[0]
        sys.modules["antenv.axon_hooks"] = mod
        antenv.axon_hooks = mod
    from antenv import axon_hooks

    if axon_hooks.get_axon_ntff_profile_hook() is None:
        from trn_agent_boot.trn_boot import _ntff_profile_via_ctypes

        axon_hooks.set_axon_ntff_profile_hook(
            _ntff_profile_via_ctypes("/opt/axon/libaxon_pjrt.so")
        )
    bass_utils.upload_artifacts = lambda tmpdir: tmpdir


def _get_nc():
    if "nc" not in _CACHE:
        _CACHE["nc"] = _build()
    return _CACHE["nc"]


def kernel(x_real: np.ndarray, x_fake: np.ndarray) -> np.ndarray:
    global LAST_EXEC_NS
    nc = _get_nc()

    in_maps = []
    for c in range(NCORES):
        sl = slice(c * BPC, (c + 1) * BPC)
        in_maps.append(
            {
                "x_real": np.ascontiguousarray(x_real[sl]).reshape(BPC * C * P, F),
                "x_fake": np.ascontiguousarray(x_fake[sl]).reshape(BPC * C * P, F),
            }
        )

    import os

    trace = bool(int(os.environ.get("KERNEL_TRACE", "0")))
    if trace:
        _register_ntff_hook()
    res = bass_utils.run_bass_kernel_spmd(
        nc, in_maps, core_ids=list(range(NCORES)), trace=trace
    )
    LAST_EXEC_NS = res.exec_time_ns
    _CACHE["last_res"] = res

    # ---- host decode ----
    # slots: 0 u2rd dual(-0.6,0.6) | 1 v2rd dual(-0.8,-0.2) | 2 v2rd dual(0.4,1.0)
    #        3 w2rd dual(-0.4,0.2) | 4 w2rd single(0.8) | 5 mx dual(0.1,0.2)
    #        6 mx dual(0.3,0.4) | 7 sign(u2) | 8 sign(v2-2) | 9 sign(w2-2)
    #        10-14 sign(mx-e), e=0.5..0.9 | 15-23 sign(q-10/k), k=1..9
    slots = np.zeros((2, NSLOT), np.float64)
    duals = np.zeros((2, 7, 2), np.float64)  # per-row unpacked dual slots 0..6
    for core_out in res.results:
        o = np.asarray(core_out["out"]).reshape(NITER, P, ACCW)[:, :, :NSLOT]
        slots[0] += o[:BPC].sum(axis=(0, 1))
        slots[1] += o[BPC:].sum(axis=(0, 1))
        pk = o[:, :, 0:7].astype(np.int64)  # exact ints in f32
        lo, hi = pk % int(PACK), pk // int(PACK)
        for t_idx, sl in ((0, slice(0, BPC)), (1, slice(BPC, NITER))):
            duals[t_idx, :, 0] += lo[sl].sum(axis=(0, 1))
            duals[t_idx, :, 1] += hi[sl].sum(axis=(0, 1))

    N = float(NPIX)  # pixels per source across all cores

    C_lt = np.zeros((2, 3, 9), np.float64)
    for t in range(2):
        NA_lo, NA_hi = duals[t, 0]                 # NA(-0.6), NA(0.6)
        NG_m8, NG_m2 = duals[t, 1]
        NG_04, NG_10 = duals[t, 2]
        NB_m4, NB_02 = duals[t, 3]
        NB_08 = duals[t, 4, 0]
        NEG = (N - slots[t, 7]) / 2.0              # #{u2 < 0}
        Gt = (N - slots[t, 8]) / 2.0               # caseG total
        Bt = (N - slots[t, 9]) / 2.0               # caseB total
        R = N - Gt - Bt
        # hue cumulative at 0.6k, k=1..9
        C_lt[t, 0, 0] = NA_hi - NEG
        for j, ng in enumerate((NG_m8, NG_m2, NG_04, NG_10)):
            C_lt[t, 0, 1 + j] = (R - NEG) + ng
        for j, nb in enumerate((NB_m4, NB_02, NB_08)):
            C_lt[t, 0, 5 + j] = (R - NEG) + Gt + nb
        C_lt[t, 0, 8] = (N - NEG) + NA_lo
        # val cumulative at 0.1k
        C_lt[t, 2, 0:4] = (duals[t, 5, 0], duals[t, 5, 1],
                           duals[t, 6, 0], duals[t, 6, 1])
        for k in range(5):
            C_lt[t, 2, 4 + k] = (N - slots[t, 10 + k]) / 2.0
        # sat cumulative: C(0.1k) = #{q > 10/k} = (N + sign_sum)/2
        for k in range(9):
            C_lt[t, 1, k] = (N + slots[t, 15 + k]) / 2.0

    hist = np.zeros((2, 3, 10), np.float64)
    hist[:, :, 0] = C_lt[:, :, 0]
    hist[:, :, 1:9] = C_lt[:, :, 1:] - C_lt[:, :, :-1]
    hist[:, :, 9] = N - C_lt[:, :, 8]

    dmean = np.abs(hist[0] - hist[1]).mean(axis=1)   # [3] = h, s, v
    loss = ALPHA * dmean[0] + BETA * dmean[1] + GAMMA * dmean[2]
    return np.asarray(loss, dtype=np.float32)


# revision 11
# speedup vs baseline: 1.1983x; 1.0410x over previous
"""ColorHistogramLoss Trainium2 kernel (8 NeuronCores, data-parallel).

Strategy: shard batch (32 -> 4 per core); each core streams its 8 images
(4 real + 4 fake) as [128, 2048] plane-triples and produces direct
cumulative histogram-edge counts; the host reassembles the three 10-bin
histograms per source and computes the scalar loss.

v2 design (VectorE+ScalarE balanced; GPSIMD unusable - SBUF port
contention with VectorE inflates both ~2.8x):
- VectorE (19 passes/iter): v=b-r, w=r-g, u=g-b, m1, mx; fused customs
  d=max(|v|,|w|,|v+w|) (1 pass), s8=mb+mg from (v,w) (1 pass, 8 ALU
  slices), case-shift customs v'=v+BIG*(1-mg), w'=w+BIG*(1-mb) straight
  from (v,w); u'=u+BIG*s8 (STT); rd=1/d; q=mx*rd.  Hue edge counts as
  dual multiply-compares (u'|v'|w')*rd vs case edges - out-of-case
  pixels sit at ~BIG*rd >= BIG-1, outside every edge, so counts are
  exactly per-case.  Two val dual-edge counts (packed c_lo+4096*c_hi).
- ScalarE (17 Sign activations/iter): val mx @ {0.5..0.9}, sat
  q=mx/d @ {10/k} (sat<c <=> q>10/k), and three mul-free hue counts:
  NEG=#{u'<0}, G-total=#{v'<2}, B-total=#{w'<2}.

Counts are exact in f32 (recip approx gives ~1e-7 slivers only); host
decode uses direct per-case cumulative counts.
"""

import sys

if "/opt/trn_rl_repo" not in sys.path:
    sys.path.insert(0, "/opt/trn_rl_repo")

import numpy as np

from concourse import bacc, mybir, tile
from concourse import bass_utils

# ---- problem constants (hardcoded; kernel.py must be self-contained) ----
B, C, H, W = 32, 3, 512, 512
NCORES = 8
BPC = B // NCORES            # batches per core
P, F = 128, 2048             # SBUF tile: one [512,512] plane = [128, 2048]
NITER = 2 * BPC              # 4 real + 4 fake plane-triple iterations
ACCW = 32                    # padded accumulator width
NPIX = B * H * W             # pixels per full histogram (all cores)
ALPHA, BETA, GAMMA = 0.3, 0.4, 0.4

AF = mybir.AluOpType
F32 = mybir.dt.float32

LAST_EXEC_NS = None
_CACHE = {}

PACK = 4096.0   # dual-count packing: accum = cnt_lo + PACK*cnt_hi (exact f32)
BIG = 1048576.0  # out-of-case shift; BIG*rd >= BIG-1 >> all hue edges

# ScalarE sign-counted edges
S_MX = (0.1, 0.2, 0.5, 0.6, 0.7, 0.8, 0.9)   # val: count mx < e (slots 10-16)
S_Q = tuple(10.0 / k for k in range(1, 10))  # sat: count q > e   (slots 17-25)

# acc slot map (see build)
NSLOT = 26


def _register_custom_ops():
    """Author + register fused DVE ops in the dve_ops registry at runtime."""
    from concourse import dve_ops
    from concourse.dve_spec import (
        C0, C1, C2, One, Spec, Src0, Src1, Zero, _has_src1, lower, maxx, minn,
    )
    from concourse.dve_uop import DveOpSpec
    from operator import add as _add

    if hasattr(dve_ops, "D3X"):
        return dve_ops

    def cref(f):
        def r(in0, in1, c0, c1, c2):
            b = f(in0, in1, c0, c1, c2).astype(np.float32)
            return b, b.reshape(b.shape[0], -1).sum(-1, keepdims=True)
        return r

    _t = Src0 + Src1

    def _mg(v, w):
        return ((v + w) < 0) & (w <= 0)

    def _mb(v, w):
        return (v >= 0) & ((v + w) >= 0)

    defs = [
        # d = max(|v|,|w|,|v+w|) == mx-mn (exact: same fl subtract results)
        ("D3X", Spec(
            body=maxx(maxx(maxx(Src0, Src1), _t), Zero - minn(minn(Src0, Src1), _t)),
            reference=lambda in0, in1, c0, c1, c2: np.maximum(
                np.maximum(np.abs(in0), np.abs(in1)), np.abs(in0 + in1)
            ).astype(np.float32),
        )),
        # s8 = mb + mg from (v,w): mb=(v>=0)&(v+w>=0), mg=(v+w<0)&(w<=0)
        ("S8C", Spec(
            body=((Src0 >= Zero) & (_t >= Zero)) + ((_t < Zero) & (Src1 <= Zero)),
            reference=lambda in0, in1, c0, c1, c2: (
                _mb(in0, in1) + _mg(in0, in1)
            ).astype(np.float32),
        )),
        # v' = v + C0*(1 - mg(v,w))
        ("VSH", Spec(
            body=Src0 + C0 * (One - ((_t < Zero) & (Src1 <= Zero))),
            reference=lambda in0, in1, c0, c1, c2: (
                in0 + c0 * (1.0 - _mg(in0, in1))
            ).astype(np.float32),
        )),
        # w' = w + C0*(1 - mb(v,w))   (Src0=v, Src1=w)
        ("WSH", Spec(
            body=Src1 + C0 * (One - ((Src0 >= Zero) & (_t >= Zero))),
            reference=lambda in0, in1, c0, c1, c2: (
                in1 + c0 * (1.0 - _mb(in0, in1))
            ).astype(np.float32),
        )),
        # dual mult-compare count: accum = #{a*b < C0} + C1*#{a*b < C2}
        ("MULCMP", Spec(
            body=(Src0 * Src1 < C0) + C1 * ((Src0 * Src1) < C2),
            accum=_add, accum_init=Zero,
            reference=cref(lambda a, b, c0, c1, c2: (
                ((a * b).astype(np.float32) < c0)
                + c1 * ((a * b).astype(np.float32) < c2)
            )),
        )),
        # mixed dual count: accum = #{a*b < C0} + C1*#{a < C2}
        ("MULCMPM", Spec(
            body=(Src0 * Src1 < C0) + C1 * (Src0 < C2),
            accum=_add, accum_init=Zero,
            reference=cref(lambda a, b, c0, c1, c2: (
                ((a * b).astype(np.float32) < c0) + c1 * (a < c2)
            )),
        )),
        # dual edge count: accum = #{x < C0} + C1*#{x < C2}
        ("LT2", Spec(
            body=(Src0 < C0) + C1 * (Src0 < C2),
            accum=_add, accum_init=Zero,
            reference=cref(lambda a, b, c0, c1, c2: (a < c0) + c1 * (a < c2)),
        )),
    ]
    for name, spec in defs:
        row = 1 + len(dve_ops.OPS)
        shas = {}
        for ver in ("v3", "v4"):
            uops = lower(spec, ver=ver)
            shas[ver] = DveOpSpec(
                name=name, opcode=row, uops=uops, rd1_en=_has_src1(spec)
            ).sha(ver)
        op = dve_ops.DveOp(name, spec, False, uops_sha=shas)
        dve_ops.OPS.append(op)
        dve_ops.CUSTOM_DVE_SPECS[name] = spec
        dve_ops._SUB_OPCODE_FOR_NAME[name] = row
        setattr(dve_ops, name, op)
    return dve_ops


def _build():
    dve_ops = _register_custom_ops()
    nc = bacc.Bacc(
        "TRN2", target_bir_lowering=False, debug=False, num_devices=NCORES
    )
    xr = nc.dram_tensor("x_real", [BPC * C * P, F], F32, kind="ExternalInput").ap()
    xf = nc.dram_tensor("x_fake", [BPC * C * P, F], F32, kind="ExternalInput").ap()
    out = nc.dram_tensor("out", [NITER * P, ACCW], F32, kind="ExternalOutput").ap()

    SIGN = mybir.ActivationFunctionType.Sign

    with tile.TileContext(nc) as tc:
        with tc.tile_pool(name="main", bufs=2) as io_pool, tc.tile_pool(
            name="tmp", bufs=1
        ) as tmp_pool:
            V, S = nc.vector, nc.scalar
            # bias tiles for ScalarE sign counts (bias = -edge)
            sbias = []
            for idx, e in enumerate(S_MX + S_Q + (0.0, 2.0)):
                bt = tmp_pool.tile([P, 1], F32, tag=f"sb{idx}", name=f"sb{idx}")
                nc.gpsimd.memset(bt[:], -e)
                sbias.append(bt)
            b_mx = sbias[0:7]
            b_q = sbias[7:16]
            b_0, b_2 = sbias[16], sbias[17]

            for it in range(NITER):
                src = xr if it < BPC else xf
                bi = it % BPC

                def plane(c):
                    qI = bi * C + c
                    return src[qI * P : (qI + 1) * P, :]

                r = io_pool.tile([P, F], F32, tag="r")
                g = io_pool.tile([P, F], F32, tag="g")
                bl = io_pool.tile([P, F], F32, tag="bl")
                # b, r first: the opening VectorE op (v = bl - r) needs them
                nc.sync.dma_start(bl[:], plane(2))
                nc.sync.dma_start(r[:], plane(0))
                nc.sync.dma_start(g[:], plane(1))

                def T(tag, bufs=1):
                    return tmp_pool.tile([P, F], F32, tag=tag, name=tag, bufs=bufs)

                acc = io_pool.tile([P, NSLOT], F32, tag="acc")
                v = T("v")
                w = T("w")
                u = T("u")
                m1 = T("m1")
                mx = T("mx", bufs=2)
                d = T("d")
                rd = T("rd")
                s8 = T("s8")
                u2 = T("u2", bufs=2)
                v2 = T("v2", bufs=2)
                w2 = T("w2", bufs=2)
                q = T("q", bufs=2)
                scr = T("scr")
                scr2 = T("scr2")

                V.tensor_tensor(v[:], bl[:], r[:], AF.subtract)
                V.tensor_tensor(w[:], r[:], g[:], AF.subtract)
                V.tensor_tensor(m1[:], r[:], g[:], AF.max)
                V.tensor_tensor(u[:], g[:], bl[:], AF.subtract)
                V.tensor_tensor(mx[:], m1[:], bl[:], AF.max)
                V._custom_dve(dve_ops.D3X, out=d[:], in0=v[:], in1=w[:])
                V.reciprocal_approx_fast(rd[:], d[:])
                V._custom_dve(dve_ops.S8C, out=s8[:], in0=v[:], in1=w[:])
                V.scalar_tensor_tensor(u2[:], s8[:], BIG, u[:], AF.mult, AF.add)
                V._custom_dve(dve_ops.VSH, out=v2[:], in0=v[:], in1=w[:], s0=BIG)
                V._custom_dve(dve_ops.WSH, out=w2[:], in0=v[:], in1=w[:], s0=BIG)
                V.tensor_tensor(q[:], mx[:], rd[:], AF.mult)

                # --- ScalarE sign counts (start on mx asap) ---
                for k in range(7):
                    S.activation(scr2[:], mx[:], SIGN, bias=b_mx[k][:],
                                 accum_out=acc[:, 10 + k : 11 + k])

                # --- VectorE dual counts ---
                def mc(src0, src1, e1, e2, slot):
                    V._custom_dve(dve_ops.MULCMP, out=scr[:], in0=src0[:],
                                  in1=src1[:], s0=e1, s1=PACK, imm2=e2,
                                  accum_out=acc[:, slot : slot + 1])

                mc(u2, rd, -0.6, 0.6, 0)
                mc(v2, rd, -0.8, -0.2, 1)
                mc(v2, rd, 0.4, 1.0, 2)
                mc(w2, rd, -0.4, 0.2, 3)
                # slot4: {w2*rd < 0.8} + PACK*{w2 < 2}  (NB(0.8) + Bt)
                V._custom_dve(dve_ops.MULCMPM, out=scr[:], in0=w2[:], in1=rd[:],
                              s0=0.8, s1=PACK, imm2=2.0, accum_out=acc[:, 4:5])
                V._custom_dve(dve_ops.LT2, out=scr[:], in0=mx[:], s0=0.3,
                              s1=PACK, imm2=0.4, accum_out=acc[:, 5:6])

                # --- remaining ScalarE sign counts ---
                for k in range(9):
                    S.activation(scr2[:], q[:], SIGN, bias=b_q[k][:],
                                 accum_out=acc[:, 17 + k : 18 + k])
                S.activation(scr2[:], u2[:], SIGN, bias=b_0[:],
                             accum_out=acc[:, 7:8])
                S.activation(scr2[:], v2[:], SIGN, bias=b_2[:],
                             accum_out=acc[:, 8:9])

                nc.sync.dma_start(out[it * P : (it + 1) * P, 0:NSLOT], acc[:, :])

    nc.compile()
    return nc


def _register_ntff_hook():
    """Register the axon NTFF profiling hook; keep artifacts local."""
    import types

    import antenv

    if "antenv.axon_hooks" not in sys.modules:
        mod = types.ModuleType("antenv.axon_hooks")
        holder = [None]
        mod.set_axon_ntff_profile_hook = lambda h: holder.__setitem__(0, h)
        mod.get_axon_ntff_profile_hook = lambda: holder[0]
        sys.modules["antenv.axon_hooks"] = mod
        antenv.axon_hooks = mod
    from antenv import axon_hooks

    if axon_hooks.get_axon_ntff_profile_hook() is None:
        from trn_agent_boot.trn_boot import _ntff_profile_via_ctypes

        axon_hooks.set_axon_ntff_profile_hook(
            _ntff_profile_via_ctypes("/opt/axon/libaxon_pjrt.so")
        )
    bass_utils.upload_artifacts = lambda tmpdir: tmpdir


def _get_nc():
    if "nc" not in _CACHE:
        _CACHE["nc"] = _build()
    return _CACHE["nc"]


def kernel(x_real: np.ndarray, x_fake: np.ndarray) -> np.ndarray:
    global LAST_EXEC_NS
    nc = _get_nc()

    in_maps = []
    for c in range(NCORES):
        sl = slice(c * BPC, (c + 1) * BPC)
        in_maps.append(
            {
                "x_real": np.ascontiguousarray(x_real[sl]).reshape(BPC * C * P, F),
                "x_fake": np.ascontiguousarray(x_fake[sl]).reshape(BPC * C * P, F),
            }
        )

    import os

    trace = bool(int(os.environ.get("KERNEL_TRACE", "0")))
    if trace:
        _register_ntff_hook()
    res = bass_utils.run_bass_kernel_spmd(
        nc, in_maps, core_ids=list(range(NCORES)), trace=trace
    )
    LAST_EXEC_NS = res.exec_time_ns
    _CACHE["last_res"] = res

    # ---- host decode ----
    # slots: 0 u2rd dual(-0.6,0.6) | 1 v2rd dual(-0.8,-0.2) | 2 v2rd dual(0.4,1.0)
    #        3 w2rd dual(-0.4,0.2) | 4 {w2rd<0.8}+P*{w2<2} | 5 mx dual(0.3,0.4)
    #        7 sign(u2) | 8 sign(v2-2) | 10-16 sign(mx-e), e in S_MX
    #        17-25 sign(q-10/k), k=1..9
    slots = np.zeros((2, NSLOT), np.float64)
    duals = np.zeros((2, 6, 2), np.float64)  # per-row unpacked dual slots 0..5
    for core_out in res.results:
        o = np.asarray(core_out["out"]).reshape(NITER, P, ACCW)[:, :, :NSLOT]
        slots[0] += o[:BPC].sum(axis=(0, 1))
        slots[1] += o[BPC:].sum(axis=(0, 1))
        pk = o[:, :, 0:6].astype(np.int64)  # exact ints in f32
        lo, hi = pk % int(PACK), pk // int(PACK)
        for t_idx, sl in ((0, slice(0, BPC)), (1, slice(BPC, NITER))):
            duals[t_idx, :, 0] += lo[sl].sum(axis=(0, 1))
            duals[t_idx, :, 1] += hi[sl].sum(axis=(0, 1))

    N = float(NPIX)  # pixels per source across all cores

    C_lt = np.zeros((2, 3, 9), np.float64)
    for t in range(2):
        NA_lo, NA_hi = duals[t, 0]                 # NA(-0.6), NA(0.6)
        NG_m8, NG_m2 = duals[t, 1]
        NG_04, NG_10 = duals[t, 2]
        NB_m4, NB_02 = duals[t, 3]
        NB_08, Bt = duals[t, 4]                    # NB(0.8), caseB total
        NEG = (N - slots[t, 7]) / 2.0              # #{u2 < 0}
        Gt = (N - slots[t, 8]) / 2.0               # caseG total
        R = N - Gt - Bt
        # hue cumulative at 0.6k, k=1..9
        C_lt[t, 0, 0] = NA_hi - NEG
        for j, ng in enumerate((NG_m8, NG_m2, NG_04, NG_10)):
            C_lt[t, 0, 1 + j] = (R - NEG) + ng
        for j, nb in enumerate((NB_m4, NB_02, NB_08)):
            C_lt[t, 0, 5 + j] = (R - NEG) + Gt + nb
        C_lt[t, 0, 8] = (N - NEG) + NA_lo
        # val cumulative at 0.1k: 0.1,0.2 signs; 0.3,0.4 dual; 0.5-0.9 signs
        C_lt[t, 2, 0] = (N - slots[t, 10]) / 2.0
        C_lt[t, 2, 1] = (N - slots[t, 11]) / 2.0
        C_lt[t, 2, 2:4] = duals[t, 5]
        for k in range(5):
            C_lt[t, 2, 4 + k] = (N - slots[t, 12 + k]) / 2.0
        # sat cumulative: C(0.1k) = #{q > 10/k} = (N + sign_sum)/2
        for k in range(9):
            C_lt[t, 1, k] = (N + slots[t, 17 + k]) / 2.0

    hist = np.zeros((2, 3, 10), np.float64)
    hist[:, :, 0] = C_lt[:, :, 0]
    hist[:, :, 1:9] = C_lt[:, :, 1:] - C_lt[:, :, :-1]
    hist[:, :, 9] = N - C_lt[:, :, 8]

    dmean = np.abs(hist[0] - hist[1]).mean(axis=1)   # [3] = h, s, v
    loss = ALPHA * dmean[0] + BETA * dmean[1] + GAMMA * dmean[2]
    return np.asarray(loss, dtype=np.float32)
